# revision 34
# baseline (speedup 1.0000x reference)
"""Causal self-attention (B=4, T=2048, C=1024, 16 heads x 64) on 8 TRN2 NeuronCores.

Sharding: tensor-parallel over heads. Core c owns heads {2c, 2c+1}:
  - w_attn column slices -> per-core QKV in transposed layout (dims on
    partitions, tokens on free dim),
  - attention in S^T form: S^T[k,q] = matmul(lhsT=kT, rhs=qT_headzeroed),
    softmax denominator via ones-columns appended to V, PV consumes exp(S^T)
    directly, partial output projection in transposed layout,
  - host sums the 8 partial projections (the TP all-reduce).

v9 (HAM-stable pipeline + fp8 QK; 390us -> 314us on HW):
  v3's chunk-boundary epilogue clustered ~10us of DVE/ScalarE PSUM-
  eviction work against ~4.5us of PE work; the resulting 300-700ns PE
  micro-stalls tripped the HAM Activity_MID window -> K=4/8 half-clock
  for 3.4-6.8us, 13x per run (~49us lost). Changes, in measured-impact
  order:
  - the deferred output projection of chunk qch-2 is spread through
    chunk qch's PV loop (one od-pair PSUM tile every ~nkb/4 k-blocks):
    a proj tile's pssp slot then has ~3.4us of PE work before reuse,
    deeply hiding its ~750ns DVE/ScalarE evictions (at the boundary the
    3-slot rotation only had ~1.1us),
  - Q/K generation in fp8e4m3 with perf_mode=DoubleRow (256-deep
    contraction, 2 MACs/cell/cycle): w_qk scaled x16 on host to clear
    the e4m3 subnormal cutoff, exp scale folds 1/256 back out. V stays
    bf16 (fp8's ~4% element error would land directly on early tokens'
    output rows; rel err 4e-3 -> 1.2e-2, gate is 2e-2),
  - PE warmup: 22 dummy matmuls at t=0 (dep: one DVE memset) warm the
    HAM clock gate while the first weight/x DMAs land,
  - proj output DMAs deferred to a later filler slot so the Sync queue
    never parks on eviction semaphores ahead of the norm-path DMAs,
  - S/exp lookahead of 3 k-blocks (pbp bufs=4) hides the exp->gpsimd
    causal-mask latency on diagonal blocks,
  - softmax reciprocal via transpose-DMA to 128 DVE lanes (a 1-lane
    [1,1024] reciprocal measures 6.5us vs ~200ns this way),
  - last batch runs its chunks in reverse (j=3..0) so the drain tail
    ends on the shortest attention chunk,
  - bf16 operands elsewhere (PSUM accum f32), f32r normalize path.
"""

import sys
import numpy as np

sys.path.insert(0, "/opt/trn_rl_repo")

B, T, C = 4, 2048, 1024
NH, HD = 16, 64
NCORES = 8
TOK = B * T                 # 8192 tokens
NCH = TOK // 512            # 16 token chunks of 512
CHB = T // 512              # 4 chunks per batch
NKB_B = T // 128            # 16 k-blocks per batch
SCALE = 1.0 / 8.0
# q,k each scaled by 16 on host so w_qk fits fp8e4m3 normals; exp folds
# the 1/256 back out
SCALE8 = SCALE / 256.0

_CACHE = {}


def _build_program():
    import concourse.tile as tile
    from concourse import bacc, mybir
    from concourse.masks import make_identity

    f32 = mybir.dt.float32
    f32r = mybir.dt.float32r
    bf16 = mybir.dt.bfloat16
    f8 = mybir.dt.float8e4

    nc = bacc.Bacc("TRN2", target_bir_lowering=False, debug=False,
                   num_devices=NCORES)

    xT = nc.dram_tensor("xT", [C, TOK], bf16, kind="ExternalInput").ap()
    xT8 = nc.dram_tensor("xT8", [C, TOK], f8, kind="ExternalInput").ap()
    wqk8 = nc.dram_tensor("wqk8", [C, 256], f8, kind="ExternalInput").ap()
    wqkv = nc.dram_tensor("wqkv", [C, 128], bf16, kind="ExternalInput").ap()
    battn = nc.dram_tensor("battn", [128, 3], f32, kind="ExternalInput").ap()
    wproj = nc.dram_tensor("wproj", [128, C], bf16, kind="ExternalInput").ap()
    bproj = nc.dram_tensor("bproj", [128, 8], f32, kind="ExternalInput").ap()
    outT = nc.dram_tensor("outT", [C, TOK], bf16, kind="ExternalOutput").ap()

    with tile.TileContext(nc) as tc:
        with tc.tile_pool(name="const", bufs=1) as const, \
             tc.tile_pool(name="resid", bufs=1) as resid, \
             tc.tile_pool(name="xin", bufs=4) as xin, \
             tc.tile_pool(name="xin8", bufs=3) as xin8, \
             tc.tile_pool(name="vsp", bufs=2) as vsp, \
             tc.tile_pool(name="pss", bufs=3, space="PSUM") as pssp, \
             tc.tile_pool(name="psy", bufs=1, space="PSUM") as psyp, \
             tc.tile_pool(name="pb", bufs=4) as pbp, \
             tc.tile_pool(name="sby", bufs=2) as sbyp, \
             tc.tile_pool(name="nrm", bufs=2) as nrm, \
             tc.tile_pool(name="ytc", bufs=3) as ytc, \
             tc.tile_pool(name="ob", bufs=2) as obp:

            xTr = xT.rearrange("(ko p) t -> p ko t", p=128)
            xT8r = xT8.rearrange("(ko p) t -> p ko t", p=128)
            outTr = outT.rearrange("(od p) t -> p od t", p=128)
            wqkvr = wqkv.rearrange("(ko p) m -> p ko m", p=128)
            wqk8r = wqk8.rearrange("(ko p) m -> p ko m", p=128)

            w_sb = const.tile([128, 8, 128], bf16, tag="w_sb")
            w8_sb = const.tile([128, 8, 256], f8, tag="w8_sb")

            def emit_A_load(ch):
                """x DMAs for one chunk, issued an iteration ahead so the
                QKV matmuls never wait on HBM."""
                x8 = xin8.tile([128, 8, 512], f8, tag="x8")
                nc.sync.dma_start(x8[:], xT8r[:, :, ch * 512:(ch + 1) * 512])
                xa = xin.tile([128, 4, 512], bf16, tag="xc")
                nc.sync.dma_start(xa[:], xTr[:, 0:4, ch * 512:(ch + 1) * 512])
                xb = xin.tile([128, 4, 512], bf16, tag="xc")
                nc.sync.dma_start(xb[:], xTr[:, 4:8, ch * 512:(ch + 1) * 512])
                return xa, xb, x8

            # startup order: qk weights + first x chunk first so the
            # first QKV matmul's deps land ASAP
            nc.sync.dma_start(w8_sb[:], wqk8r[:])
            pend_x = []      # [(xc, x8)] prefetched x chunks
            pend_x.append(emit_A_load(0))
            bias = const.tile([128, 11], f32, tag="bias")
            battn_sb = bias[:, 0:3]
            bp_sb = bias[:, 3:11]
            nc.sync.dma_start(battn_sb, battn[:])
            nc.sync.dma_start(w_sb[:], wqkvr[:])
            pend_x.append(emit_A_load(1))
            nc.sync.dma_start(bp_sb, bproj[:])
            wp_sb = const.tile([128, C], bf16, tag="wp")
            nc.sync.dma_start(wp_sb[:], wproj[:])

            ident = const.tile([128, 128], f32, tag="ident")
            make_identity(nc, ident)
            ones_r = const.tile([1, 128], f32r, tag="ones")
            nc.gpsimd.memset(ones_r.bitcast(f32), 1.0)

            # resident activations
            kT = resid.tile([128, NCH, 512], bf16, tag="kT")
            # qz: both heads' q in one tile; [:, :, 0:512] = head0 slots
            # (partitions 64:128 zero), [:, :, 512:1024] = head1 slots
            # (partitions 0:64 zero) -> one 1024-row S matmul per k-block
            qz = resid.tile([128, NCH, 1024], bf16, tag="qz")
            vpr = resid.tile([128, 64, 132], bf16, tag="vpr")
            vprv = vpr.rearrange("p kb (h c) -> p kb h c", c=66)

            # PE warmup: dummy matmuls whose only dep is a DVE memset.
            # They run while the startup DMAs land, pushing the HAM clock
            # gate to K=8/8 before the first real matmul.
            warm = const.tile([128, 512], bf16, tag="warm")
            nc.vector.memset(warm[:], 0.0)
            for wi in range(2):
                ws = pssp.tile([128, 1024], f32, tag="s")
                for wj in range(13):
                    nc.tensor.matmul(ws[:, 0:512], warm[:, 0:128], warm[:],
                                     start=True, stop=True)

            nc.vector.memset(qz[64:128, :, 0:512], 0.0)
            nc.gpsimd.memset(qz[0:64, :, 512:1024], 0.0)
            nc.gpsimd.memset(vprv[:, :, :, 64:66], 1.0)

            def emit_A_mm_qk(ch, xa, xb, x8):
                """Q/K matmuls (fp8 DoubleRow: 256-deep contraction, 2
                MACs/cell/cycle -> half the matmuls of bf16) + q/k
                evictions for one 512-token chunk."""
                tQK = pssp.tile([128, 1024], f32, tag="s")
                for half, m in ((0, 0), (1, 1)):
                    for k2 in range(4):
                        nc.tensor.matmul(
                            tQK[:, half * 512:(half + 1) * 512],
                            w8_sb[:, 2 * k2:2 * k2 + 2,
                                  m * 128:(m + 1) * 128],
                            x8[:, 2 * k2:2 * k2 + 2, :],
                            start=(k2 == 0), stop=(k2 == 3),
                            perf_mode=mybir.MatmulPerfMode.DoubleRow)
                nc.vector.tensor_scalar_add(qz[0:64, ch, 0:512],
                                            tQK[0:64, 0:512], battn_sb[0:64, 0:1])
                nc.vector.tensor_scalar_add(qz[64:128, ch, 512:1024],
                                            tQK[64:128, 0:512], battn_sb[64:128, 0:1])
                nc.scalar.activation(kT[:, ch, :], tQK[:, 512:1024],
                                     mybir.ActivationFunctionType.Identity,
                                     bias=battn_sb[:, 1:2])

            def emit_A_mm_v(ch, xa, xb, x8):
                """V matmuls + v eviction for one 512-token chunk.
                V stays bf16: fp8's ~4% element error would land directly
                on the output rows of early (peaked-softmax) tokens.
                V transposes are deferred (emit_A_transp)."""
                tV = pssp.tile([128, 1024], f32, tag="s")
                for ko in range(8):
                    xsrc = xa if ko < 4 else xb
                    nc.tensor.matmul(tV[:, 0:512],
                                     w_sb[:, ko, :],
                                     xsrc[:, ko % 4, :],
                                     start=(ko == 0), stop=(ko == 7))
                vs = vsp.tile([128, 512], f32, tag="vs")
                nc.scalar.activation(vs[:], tV[:, 0:512],
                                     mybir.ActivationFunctionType.Identity,
                                     bias=battn_sb[:, 2:3])
                return ch, vs, tV

            def emit_A_transp(ch, vs, tV):
                # PE transposes into the back half of tV (vs eviction has
                # long completed by the time the PE reaches these)
                tVt = tV.rearrange("p (tb d) -> p tb d", d=128)
                for t in range(4):
                    nc.tensor.transpose(tVt[:, 4 + t, :],
                                        vs[:, t * 128:(t + 1) * 128], ident)
                    nc.vector.tensor_copy(
                        vprv[:, ch * 4 + t, :, 0:64],
                        tVt[:, 4 + t, :].rearrange("p (h d) -> p h d", d=64))

            def emit_S(b, j, qch, kb):
                vstart = max(0, kb * 128 - j * 512)
                kch = b * CHB + kb // 4
                ksub = (kb % 4) * 128
                # matmul moving dim is capped at 512 (one PSUM bank), so
                # one matmul per head into halves of a shared tile; the
                # merged Exp then covers both halves in one activation
                s = pssp.tile([128, 1024], f32, tag="s")
                for h in range(2):
                    nc.tensor.matmul(
                        s[:, h * 512 + vstart:(h + 1) * 512],
                        kT[:, kch, ksub:ksub + 128],
                        qz[:, qch, h * 512 + vstart:(h + 1) * 512],
                        start=True, stop=True)
                return s, vstart

            def emit_exp(j, kb, s, vstart):
                p = pbp.tile([128, 1024], bf16, tag="p")
                pv = p.rearrange("p (h q) -> p h q", q=512)
                sv = s.rearrange("p (h q) -> p h q", q=512)
                nc.scalar.activation(pv[:, :, vstart:], sv[:, :, vstart:],
                                     mybir.ActivationFunctionType.Exp, scale=SCALE8)
                if kb >= 4 * j:
                    # inline causal mask on GpSimd: keep where k <= q
                    nc.gpsimd.affine_select(
                        out=pv[:, :, vstart:vstart + 128],
                        in_=pv[:, :, vstart:vstart + 128],
                        compare_op=mybir.AluOpType.is_gt,
                        fill=0.0, base=1,
                        pattern=[[0, 2], [1, 128]], channel_multiplier=-1,
                    )
                return p, vstart

            def emit_epilogue(qch, psY):
                # cols 0:1024 hold psY (row 64 = softmax denominators);
                # 1-lane reciprocal into cols 1024:2048 replaces the old
                # transpose-DMA -> 128-lane recip -> DMA-back chain
                # (saves ~4us of DMA latency per chunk)
                # the transpose-DMA puts the 1024 denominators on 128 DVE
                # lanes: a [1,1024] 1-lane reciprocal measures ~6.5us vs
                # ~200ns here. The DMA latency is hidden by the one-chunk
                # norm deferral.
                sbY = sbyp.tile([66, 1024], f32, tag="sby")
                nc.vector.tensor_copy(sbY[:], psY[0:66, :, :])
                sc = nrm.tile([128, 16], f32, tag="sc")
                nc.sync.dma_start(sc[:, 0:8], sbY[64:65, :])
                nc.vector.reciprocal(sc[:, 8:16], sc[:, 0:8])
                rrt = nrm.tile([1, 1024], f32r, tag="rr")
                nc.sync.dma_start(rrt[0:1, :], sc[:, 8:16].bitcast(f32r))
                return (sbY, rrt)

            def emit_norm(qch, sbY, rrt):
                rr = rrt[0:1, :]
                r = pssp.tile([128, 1024], f32, tag="s")
                nc.tensor.matmul(r[:, 0:512], ones_r[0:1, :], rr[:, 0:512],
                                 start=True, stop=True)
                nc.tensor.matmul(r[:, 512:1024], ones_r[0:1, :], rr[:, 512:1024],
                                 start=True, stop=True)
                yTch = ytc.tile([128, 512], bf16, tag="yt")
                nc.vector.tensor_mul(yTch[0:64, :], sbY[0:64, 0:512], r[0:64, 0:512])
                yst = nrm.tile([64, 512], bf16, tag="yst")
                nc.vector.tensor_mul(yst[:], sbY[0:64, 512:1024], r[0:64, 512:1024])
                nc.sync.dma_start(yTch[64:128, :], yst[:])
                return yTch

            def emit_proj_group(qch, yTch, oSb, g, sc_both=False):
                # one PSUM tile covering od pair (2g, 2g+1); evictions
                # split DVE/ScalarE so no single engine's boundary burst
                # exceeds the PE's boundary matmul work. sc_both routes
                # both to ScalarE (used when DVE is busy with the
                # epilogue copy+reciprocal and ScalarE has no kv work).
                tP = pssp.tile([128, 1024], f32, tag="s")
                for h in range(2):
                    od = 2 * g + h
                    nc.tensor.matmul(tP[:, h * 512:(h + 1) * 512],
                                     wp_sb[:, od * 128:(od + 1) * 128],
                                     yTch[:], start=True, stop=True)
                    src = tP[:, h * 512:(h + 1) * 512]
                    if h == 0 and not sc_both:
                        nc.vector.tensor_scalar_add(oSb[:, od, :], src,
                                                    bp_sb[:, od:od + 1])
                    else:
                        # GpSimd cannot read PSUM on TRN2
                        nc.scalar.activation(
                            oSb[:, od, :], src,
                            mybir.ActivationFunctionType.Identity,
                            bias=bp_sb[:, od:od + 1])
                if g % 2 == 1:
                    # deferred: issuing this DMA immediately would park the
                    # Sync queue on the eviction semaphores and delay the
                    # norm-path DMAs queued behind it
                    def odma(qch=qch, oSb=oSb, g=g):
                        nc.sync.dma_start(
                            outTr[:, 2 * g - 2:2 * g + 2,
                                  qch * 512:(qch + 1) * 512],
                            oSb[:, 2 * g - 2:2 * g + 2, :])
                    return odma
                return None

            def emit_proj(qch, yTch):
                oSb = obp.tile([128, 8, 512], bf16, tag="o")
                for g in range(4):
                    od = emit_proj_group(qch, yTch, oSb, g)
                    if od:
                        od()

            # ---------------- fused schedule ----------------
            # QKV for batch 0 up front (transposes deferred by one chunk
            # so the v eviction is covered); then per attention chunk of
            # batch b, interleave one QKV chunk of batch b+1.
            pend_tr = []     # [(ch, vs, tV)] QKV chunks awaiting transposes
            pend_norm = []   # [(qch, sbY)]
            pend_proj = []   # [(qch, yTch)]
            pend_odma = []   # deferred projection output DMAs
            for ch in range(CHB):
                xab = pend_x.pop(0)
                emit_A_mm_qk(ch, *xab)
                ep = emit_A_mm_v(ch, *xab)
                if ch + 2 < CHB + 1:
                    pend_x.append(emit_A_load(ch + 2))
                if pend_tr:
                    emit_A_transp(*pend_tr.pop(0))
                pend_tr.append(ep)
            emit_A_transp(*pend_tr.pop(0))

            for b in range(B):
                js = range(CHB) if b + 1 < B else range(CHB - 1, -1, -1)
                for j in js:
                    qch = b * CHB + j
                    psY = psyp.tile([128, 2, 512], f32, tag="y")
                    nkb = 4 * j + 4

                    # spread the deferred projection of chunk qch-2 through
                    # this chunk's PV loop (one od-pair group every ~nkb/4
                    # k-blocks): the pssp slot that a proj tile occupies is
                    # then not rewritten for ~2 k-block iterations (~3.4us
                    # of PE work), deeply hiding its ~750ns PSUM evictions.
                    # Emitting them at the boundary instead stalled the PE
                    # ~500-700ns x3 per chunk and tripped the HAM gate.
                    can_pop = len(pend_proj) >= (2 if b + 1 < B else 1)
                    oproj = pend_proj.pop(0) if can_pop else None
                    if oproj:
                        oSb = obp.tile([128, 8, 512], bf16, tag="o")
                        step = max(1, nkb // 4)
                        fill_at = {step - 1 + i * step: i for i in range(4)}

                    # lookahead 3: the diagonal-block causal masks add
                    # exp->gpsimd latency; at depth 2 the short (j=0)
                    # chunks stalled on them
                    sq = [emit_exp(j, kb0, *emit_S(b, j, qch, kb0))
                          for kb0 in range(min(3, nkb))]
                    for kb in range(nkb):
                        p, vstart = sq.pop(0)
                        pv = p.rearrange("p (h q) -> p h q", q=512)
                        gkb = b * NKB_B + kb
                        for h in range(2):
                            nc.tensor.matmul(psY[0:66, h, vstart:],
                                             vprv[:, gkb, h, :],
                                             pv[:, h, vstart:],
                                             start=(kb == 0), stop=(kb == nkb - 1))
                        if oproj and kb in fill_at:
                            if pend_odma:
                                pend_odma.pop(0)()
                            od = emit_proj_group(oproj[0], oproj[1], oSb,
                                                 fill_at[kb])
                            if od:
                                pend_odma.append(od)
                        if kb + 3 < nkb:
                            sq.append(emit_exp(j, kb + 3,
                                               *emit_S(b, j, qch, kb + 3)))

                    ep = emit_epilogue(qch, psY)
                    # prefetch x for the NEXT iteration's QKV chunk
                    if b + 1 < B:
                        nxt = b * CHB + CHB + j + 1
                        if nxt < NCH:
                            pend_x.append(emit_A_load(nxt))
                    # boundary: next batch's QKV, V transposes, and the
                    # deferred norm of chunk qch-1
                    xab = pend_x.pop(0) if b + 1 < B else None
                    if xab:
                        emit_A_mm_qk(b * CHB + CHB + j, *xab)
                    at = (emit_A_mm_v(b * CHB + CHB + j, *xab)
                          if xab else None)
                    if at:
                        emit_A_transp(*at)
                    if pend_norm:
                        pq, psbY, prrt = pend_norm.pop(0)
                        pend_proj.append((pq, emit_norm(pq, psbY, prrt)))
                    pend_norm.append((qch, *ep))
            # drain
            for od in pend_odma:
                od()
            pend_odma = []
            while pend_proj:
                emit_proj(*pend_proj.pop(0))
            for pq, psbY, prrt in pend_norm:
                emit_proj(pq, emit_norm(pq, psbY, prrt))

    nc.compile()
    return nc


def _get_program():
    if "nc" not in _CACHE:
        _CACHE["nc"] = _build_program()
    return _CACHE["nc"]


def kernel(x, w_attn, b_attn, w_proj, b_proj, _trace=False):
    import ml_dtypes
    from concourse.bass_utils import run_bass_kernel_spmd

    bf16 = ml_dtypes.bfloat16
    nc = _get_program()

    x = np.asarray(x, dtype=np.float32)
    w_attn = np.asarray(w_attn, dtype=np.float32)
    b_attn = np.asarray(b_attn, dtype=np.float32)
    w_proj = np.asarray(w_proj, dtype=np.float32)
    b_proj = np.asarray(b_proj, dtype=np.float32)

    f8 = ml_dtypes.float8_e4m3  # IEEE-style e4m3: matches TRN FP8_EXP4
    xTf = x.reshape(TOK, C).T
    xT_np = np.ascontiguousarray(xTf.astype(bf16))
    xT8_np = np.ascontiguousarray(xTf.astype(f8))

    in_maps = []
    for c in range(NCORES):
        lo, hi = c * 128, (c + 1) * 128
        wq = w_attn[:, lo:hi]
        wk = w_attn[:, C + lo:C + hi]
        wv = w_attn[:, 2 * C + lo:2 * C + hi]
        # q,k weights x16 into fp8 (w~0.02*N(0,1) would land in e4m3's
        # subnormal range unscaled); exp scale folds the 256 back out
        wqk8_np = np.ascontiguousarray(
            (np.concatenate([wq, wk], axis=1) * 16.0).astype(f8))
        wqkv_np = np.ascontiguousarray(wv.astype(bf16))
        bq = b_attn[lo:hi] * 16.0
        bk = b_attn[C + lo:C + hi] * 16.0
        bv = b_attn[2 * C + lo:2 * C + hi]
        battn_np = np.ascontiguousarray(np.stack([bq, bk, bv], axis=1))  # [128, 3]
        wproj_np = np.ascontiguousarray(w_proj[lo:hi, :].astype(bf16))
        if c == 0:
            bproj_np = np.ascontiguousarray(b_proj.reshape(8, 128).T)
        else:
            bproj_np = np.zeros((128, 8), dtype=np.float32)
        in_maps.append({
            "xT": xT_np,
            "xT8": xT8_np,
            "wqk8": wqk8_np,
            "wqkv": wqkv_np,
            "battn": battn_np,
            "wproj": wproj_np,
            "bproj": bproj_np,
        })

    res = run_bass_kernel_spmd(nc, in_maps, core_ids=list(range(NCORES)),
                               trace=_trace)
    acc = res.results[0]["outT"].astype(np.float32)
    for c in range(1, NCORES):
        acc += res.results[c]["outT"].astype(np.float32)
    out = np.ascontiguousarray(acc.T).reshape(B, T, C)
    if _trace:
        kernel.last_exec_time_ns = res.exec_time_ns
        kernel.last_scope_times = res.per_core_scope_times
        kernel.last_trace = res.instructions_and_trace
    return out


# revision 35
# speedup vs baseline: 1.0358x; 1.0358x over previous
"""Causal self-attention (B=4, T=2048, C=1024, 16 heads x 64) on 8 TRN2 NeuronCores.

Sharding: tensor-parallel over heads. Core c owns heads {2c, 2c+1}:
  - w_attn column slices -> per-core QKV in transposed layout (dims on
    partitions, tokens on free dim),
  - attention in S^T form: S^T[k,q] = matmul(lhsT=kT, rhs=qT_headzeroed),
    softmax denominator via ones-columns appended to V, PV consumes exp(S^T)
    directly, partial output projection in transposed layout,
  - host sums the 8 partial projections (the TP all-reduce).

v9 (HAM-stable pipeline + fp8 QK; 390us -> 314us on HW):
  v3's chunk-boundary epilogue clustered ~10us of DVE/ScalarE PSUM-
  eviction work against ~4.5us of PE work; the resulting 300-700ns PE
  micro-stalls tripped the HAM Activity_MID window -> K=4/8 half-clock
  for 3.4-6.8us, 13x per run (~49us lost). Changes, in measured-impact
  order:
  - the deferred output projection of chunk qch-2 is spread through
    chunk qch's PV loop (one od-pair PSUM tile every ~nkb/4 k-blocks):
    a proj tile's pssp slot then has ~3.4us of PE work before reuse,
    deeply hiding its ~750ns DVE/ScalarE evictions (at the boundary the
    3-slot rotation only had ~1.1us),
  - Q/K generation in fp8e4m3 with perf_mode=DoubleRow (256-deep
    contraction, 2 MACs/cell/cycle): w_qk scaled x16 on host to clear
    the e4m3 subnormal cutoff, exp scale folds 1/256 back out. V stays
    bf16 (fp8's ~4% element error would land directly on early tokens'
    output rows; rel err 4e-3 -> 1.2e-2, gate is 2e-2),
  - PE warmup: 22 dummy matmuls at t=0 (dep: one DVE memset) warm the
    HAM clock gate while the first weight/x DMAs land,
  - proj output DMAs deferred to a later filler slot so the Sync queue
    never parks on eviction semaphores ahead of the norm-path DMAs,
  - S/exp lookahead of 3 k-blocks (pbp bufs=4) hides the exp->gpsimd
    causal-mask latency on diagonal blocks,
  - softmax reciprocal via transpose-DMA to 128 DVE lanes (a 1-lane
    [1,1024] reciprocal measures 6.5us vs ~200ns this way),
  - last batch runs its chunks in reverse (j=3..0) so the drain tail
    ends on the shortest attention chunk,
  - bf16 operands elsewhere (PSUM accum f32), f32r normalize path.
"""

import sys
import numpy as np

sys.path.insert(0, "/opt/trn_rl_repo")

B, T, C = 4, 2048, 1024
NH, HD = 16, 64
NCORES = 8
TOK = B * T                 # 8192 tokens
NCH = TOK // 512            # 16 token chunks of 512
CHB = T // 512              # 4 chunks per batch
NKB_B = T // 128            # 16 k-blocks per batch
SCALE = 1.0 / 8.0
# q,k each scaled by 16 on host so w_qk fits fp8e4m3 normals; exp folds
# the 1/256 back out
SCALE8 = SCALE / 256.0

_CACHE = {}


def _build_program():
    import concourse.tile as tile
    from concourse import bacc, mybir
    from concourse.masks import make_identity

    f32 = mybir.dt.float32
    f32r = mybir.dt.float32r
    bf16 = mybir.dt.bfloat16
    f8 = mybir.dt.float8e4

    nc = bacc.Bacc("TRN2", target_bir_lowering=False, debug=False,
                   num_devices=NCORES)

    xT = nc.dram_tensor("xT", [C, TOK], bf16, kind="ExternalInput").ap()
    xT8 = nc.dram_tensor("xT8", [C, TOK], f8, kind="ExternalInput").ap()
    wqk8 = nc.dram_tensor("wqk8", [C, 256], f8, kind="ExternalInput").ap()
    wqkv = nc.dram_tensor("wqkv", [C, 128], bf16, kind="ExternalInput").ap()
    battn = nc.dram_tensor("battn", [128, 3], f32, kind="ExternalInput").ap()
    wproj = nc.dram_tensor("wproj", [128, C], bf16, kind="ExternalInput").ap()
    bproj = nc.dram_tensor("bproj", [128, 8], f32, kind="ExternalInput").ap()
    outT = nc.dram_tensor("outT", [C, TOK], bf16, kind="ExternalOutput").ap()

    with tile.TileContext(nc) as tc:
        with tc.tile_pool(name="const", bufs=1) as const, \
             tc.tile_pool(name="resid", bufs=1) as resid, \
             tc.tile_pool(name="xin", bufs=4) as xin, \
             tc.tile_pool(name="xin8", bufs=3) as xin8, \
             tc.tile_pool(name="vsp", bufs=2) as vsp, \
             tc.tile_pool(name="pss", bufs=3, space="PSUM") as pssp, \
             tc.tile_pool(name="psy", bufs=1, space="PSUM") as psyp, \
             tc.tile_pool(name="pb", bufs=4) as pbp, \
             tc.tile_pool(name="sby", bufs=2) as sbyp, \
             tc.tile_pool(name="nrm", bufs=2) as nrm, \
             tc.tile_pool(name="ytc", bufs=3) as ytc, \
             tc.tile_pool(name="ob", bufs=2) as obp:

            xTr = xT.rearrange("(ko p) t -> p ko t", p=128)
            xT8r = xT8.rearrange("(ko p) t -> p ko t", p=128)
            outTr = outT.rearrange("(od p) t -> p od t", p=128)
            wqkvr = wqkv.rearrange("(ko p) m -> p ko m", p=128)
            wqk8r = wqk8.rearrange("(ko p) m -> p ko m", p=128)

            w_sb = const.tile([128, 8, 128], bf16, tag="w_sb")
            w8_sb = const.tile([128, 8, 256], f8, tag="w8_sb")

            def emit_A_load(ch):
                """x DMAs for one chunk, issued an iteration ahead so the
                QKV matmuls never wait on HBM."""
                x8 = xin8.tile([128, 8, 512], f8, tag="x8")
                nc.sync.dma_start(x8[:], xT8r[:, :, ch * 512:(ch + 1) * 512])
                xa = xin.tile([128, 4, 512], bf16, tag="xc")
                nc.sync.dma_start(xa[:], xTr[:, 0:4, ch * 512:(ch + 1) * 512])
                xb = xin.tile([128, 4, 512], bf16, tag="xc")
                nc.sync.dma_start(xb[:], xTr[:, 4:8, ch * 512:(ch + 1) * 512])
                return xa, xb, x8

            # startup order: qk weights + first x chunk first so the
            # first QKV matmul's deps land ASAP
            nc.sync.dma_start(w8_sb[:], wqk8r[:])
            pend_x = []      # [(xc, x8)] prefetched x chunks
            pend_x.append(emit_A_load(0))
            bias = const.tile([128, 11], f32, tag="bias")
            battn_sb = bias[:, 0:3]
            bp_sb = bias[:, 3:11]
            nc.sync.dma_start(battn_sb, battn[:])
            nc.sync.dma_start(w_sb[:], wqkvr[:])
            pend_x.append(emit_A_load(1))
            nc.sync.dma_start(bp_sb, bproj[:])
            wp_sb = const.tile([128, C], bf16, tag="wp")
            nc.sync.dma_start(wp_sb[:], wproj[:])

            ident = const.tile([128, 128], f32, tag="ident")
            make_identity(nc, ident)
            ones_r = const.tile([1, 128], f32r, tag="ones")
            nc.gpsimd.memset(ones_r.bitcast(f32), 1.0)

            # resident activations
            kT = resid.tile([128, NCH, 512], bf16, tag="kT")
            # qz: both heads' q in one tile; [:, :, 0:512] = head0 slots
            # (partitions 64:128 zero), [:, :, 512:1024] = head1 slots
            # (partitions 0:64 zero) -> one 1024-row S matmul per k-block
            qz = resid.tile([128, NCH, 1024], bf16, tag="qz")
            vpr = resid.tile([128, 64, 132], bf16, tag="vpr")
            vprv = vpr.rearrange("p kb (h c) -> p kb h c", c=66)

            # PE warmup: dummy matmuls whose only dep is a DVE memset.
            # They run while the startup DMAs land, pushing the HAM clock
            # gate to K=8/8 before the first real matmul.
            warm = const.tile([128, 512], bf16, tag="warm")
            nc.vector.memset(warm[:], 0.0)
            for wi in range(2):
                ws = pssp.tile([128, 1024], f32, tag="s")
                for wj in range(11):
                    nc.tensor.matmul(ws[:, 0:512], warm[:, 0:128], warm[:],
                                     start=True, stop=True)

            nc.vector.memset(qz[64:128, :, 0:512], 0.0)
            nc.gpsimd.memset(qz[0:64, :, 512:1024], 0.0)
            nc.gpsimd.memset(vprv[:, :, :, 64:66], 1.0)

            def emit_A_mm_qk(ch, xa, xb, x8):
                """Q/K matmuls (fp8 DoubleRow: 256-deep contraction, 2
                MACs/cell/cycle -> half the matmuls of bf16) + q/k
                evictions for one 512-token chunk."""
                tQK = pssp.tile([128, 1024], f32, tag="s")
                for half, m in ((0, 0), (1, 1)):
                    for k2 in range(4):
                        nc.tensor.matmul(
                            tQK[:, half * 512:(half + 1) * 512],
                            w8_sb[:, 2 * k2:2 * k2 + 2,
                                  m * 128:(m + 1) * 128],
                            x8[:, 2 * k2:2 * k2 + 2, :],
                            start=(k2 == 0), stop=(k2 == 3),
                            perf_mode=mybir.MatmulPerfMode.DoubleRow)
                nc.vector.tensor_scalar_add(qz[0:64, ch, 0:512],
                                            tQK[0:64, 0:512], battn_sb[0:64, 0:1])
                nc.vector.tensor_scalar_add(qz[64:128, ch, 512:1024],
                                            tQK[64:128, 0:512], battn_sb[64:128, 0:1])
                nc.scalar.activation(kT[:, ch, :], tQK[:, 512:1024],
                                     mybir.ActivationFunctionType.Identity,
                                     bias=battn_sb[:, 1:2])

            def emit_A_mm_v(ch, xa, xb, x8):
                """V matmuls + v eviction for one 512-token chunk.
                V stays bf16: fp8's ~4% element error would land directly
                on the output rows of early (peaked-softmax) tokens.
                V transposes are deferred (emit_A_transp)."""
                tV = pssp.tile([128, 1024], f32, tag="s")
                for ko in range(8):
                    xsrc = xa if ko < 4 else xb
                    nc.tensor.matmul(tV[:, 0:512],
                                     w_sb[:, ko, :],
                                     xsrc[:, ko % 4, :],
                                     start=(ko == 0), stop=(ko == 7))
                vs = vsp.tile([128, 512], f32, tag="vs")
                nc.scalar.activation(vs[:], tV[:, 0:512],
                                     mybir.ActivationFunctionType.Identity,
                                     bias=battn_sb[:, 2:3])
                return ch, vs, tV

            def emit_A_transp(ch, vs, tV):
                # PE transposes into the back half of tV (vs eviction has
                # long completed by the time the PE reaches these)
                tVt = tV.rearrange("p (tb d) -> p tb d", d=128)
                for t in range(4):
                    nc.tensor.transpose(tVt[:, 4 + t, :],
                                        vs[:, t * 128:(t + 1) * 128], ident)
                    nc.vector.tensor_copy(
                        vprv[:, ch * 4 + t, :, 0:64],
                        tVt[:, 4 + t, :].rearrange("p (h d) -> p h d", d=64))

            def emit_S(b, j, qch, kb):
                vstart = max(0, kb * 128 - j * 512)
                kch = b * CHB + kb // 4
                ksub = (kb % 4) * 128
                # matmul moving dim is capped at 512 (one PSUM bank), so
                # one matmul per head into halves of a shared tile; the
                # merged Exp then covers both halves in one activation
                s = pssp.tile([128, 1024], f32, tag="s")
                for h in range(2):
                    nc.tensor.matmul(
                        s[:, h * 512 + vstart:(h + 1) * 512],
                        kT[:, kch, ksub:ksub + 128],
                        qz[:, qch, h * 512 + vstart:(h + 1) * 512],
                        start=True, stop=True)
                return s, vstart

            def emit_exp(j, kb, s, vstart):
                p = pbp.tile([128, 1024], bf16, tag="p")
                pv = p.rearrange("p (h q) -> p h q", q=512)
                sv = s.rearrange("p (h q) -> p h q", q=512)
                nc.scalar.activation(pv[:, :, vstart:], sv[:, :, vstart:],
                                     mybir.ActivationFunctionType.Exp, scale=SCALE8)
                if kb >= 4 * j:
                    # inline causal mask on GpSimd: keep where k <= q
                    nc.gpsimd.affine_select(
                        out=pv[:, :, vstart:vstart + 128],
                        in_=pv[:, :, vstart:vstart + 128],
                        compare_op=mybir.AluOpType.is_gt,
                        fill=0.0, base=1,
                        pattern=[[0, 2], [1, 128]], channel_multiplier=-1,
                    )
                return p, vstart

            def emit_epilogue(qch, psY):
                # cols 0:1024 hold psY (row 64 = softmax denominators);
                # 1-lane reciprocal into cols 1024:2048 replaces the old
                # transpose-DMA -> 128-lane recip -> DMA-back chain
                # (saves ~4us of DMA latency per chunk)
                # the transpose-DMA puts the 1024 denominators on 128 DVE
                # lanes: a [1,1024] 1-lane reciprocal measures ~6.5us vs
                # ~200ns here. The DMA latency is hidden by the one-chunk
                # norm deferral.
                sbY = sbyp.tile([66, 1024], f32, tag="sby")
                nc.vector.tensor_copy(sbY[:], psY[0:66, :, :])
                sc = nrm.tile([128, 16], f32, tag="sc")
                nc.sync.dma_start(sc[:, 0:8], sbY[64:65, :])
                nc.vector.reciprocal(sc[:, 8:16], sc[:, 0:8])
                rrt = nrm.tile([1, 1024], f32r, tag="rr")
                nc.sync.dma_start(rrt[0:1, :], sc[:, 8:16].bitcast(f32r))
                return (sbY, rrt)

            def emit_norm(qch, sbY, rrt):
                rr = rrt[0:1, :]
                r = pssp.tile([128, 1024], f32, tag="s")
                nc.tensor.matmul(r[:, 0:512], ones_r[0:1, :], rr[:, 0:512],
                                 start=True, stop=True)
                nc.tensor.matmul(r[:, 512:1024], ones_r[0:1, :], rr[:, 512:1024],
                                 start=True, stop=True)
                yTch = ytc.tile([128, 512], bf16, tag="yt")
                nc.vector.tensor_mul(yTch[0:64, :], sbY[0:64, 0:512], r[0:64, 0:512])
                yst = nrm.tile([64, 512], bf16, tag="yst")
                nc.vector.tensor_mul(yst[:], sbY[0:64, 512:1024], r[0:64, 512:1024])
                nc.sync.dma_start(yTch[64:128, :], yst[:])
                return yTch

            def emit_proj_group(qch, yTch, oSb, g, sc_both=False):
                # one PSUM tile covering od pair (2g, 2g+1); evictions
                # split DVE/ScalarE so no single engine's boundary burst
                # exceeds the PE's boundary matmul work. sc_both routes
                # both to ScalarE (used when DVE is busy with the
                # epilogue copy+reciprocal and ScalarE has no kv work).
                tP = pssp.tile([128, 1024], f32, tag="s")
                for h in range(2):
                    od = 2 * g + h
                    nc.tensor.matmul(tP[:, h * 512:(h + 1) * 512],
                                     wp_sb[:, od * 128:(od + 1) * 128],
                                     yTch[:], start=True, stop=True)
                    src = tP[:, h * 512:(h + 1) * 512]
                    if h == 0 and not sc_both:
                        nc.vector.tensor_scalar_add(oSb[:, od, :], src,
                                                    bp_sb[:, od:od + 1])
                    else:
                        # GpSimd cannot read PSUM on TRN2
                        nc.scalar.activation(
                            oSb[:, od, :], src,
                            mybir.ActivationFunctionType.Identity,
                            bias=bp_sb[:, od:od + 1])
                if g % 2 == 1:
                    # deferred: issuing this DMA immediately would park the
                    # Sync queue on the eviction semaphores and delay the
                    # norm-path DMAs queued behind it
                    def odma(qch=qch, oSb=oSb, g=g):
                        nc.sync.dma_start(
                            outTr[:, 2 * g - 2:2 * g + 2,
                                  qch * 512:(qch + 1) * 512],
                            oSb[:, 2 * g - 2:2 * g + 2, :])
                    return odma
                return None

            def emit_proj(qch, yTch):
                oSb = obp.tile([128, 8, 512], bf16, tag="o")
                for g in range(4):
                    od = emit_proj_group(qch, yTch, oSb, g)
                    if od:
                        od()

            # ---------------- fused schedule ----------------
            # QKV for batch 0 up front (transposes deferred by one chunk
            # so the v eviction is covered); then per attention chunk of
            # batch b, interleave one QKV chunk of batch b+1.
            pend_tr = []     # [(ch, vs, tV)] QKV chunks awaiting transposes
            pend_norm = []   # [(qch, sbY)]
            pend_proj = []   # [(qch, yTch)]
            pend_odma = []   # deferred projection output DMAs
            for ch in range(CHB):
                xab = pend_x.pop(0)
                emit_A_mm_qk(ch, *xab)
                ep = emit_A_mm_v(ch, *xab)
                if ch + 2 < CHB + 1:
                    pend_x.append(emit_A_load(ch + 2))
                if pend_tr:
                    emit_A_transp(*pend_tr.pop(0))
                pend_tr.append(ep)
            emit_A_transp(*pend_tr.pop(0))

            for b in range(B):
                js = range(CHB) if b + 1 < B else range(CHB - 1, -1, -1)
                for j in js:
                    qch = b * CHB + j
                    psY = psyp.tile([128, 2, 512], f32, tag="y")
                    nkb = 4 * j + 4

                    # spread the deferred projection of chunk qch-2 through
                    # this chunk's PV loop (one od-pair group every ~nkb/4
                    # k-blocks): the pssp slot that a proj tile occupies is
                    # then not rewritten for ~2 k-block iterations (~3.4us
                    # of PE work), deeply hiding its ~750ns PSUM evictions.
                    # Emitting them at the boundary instead stalled the PE
                    # ~500-700ns x3 per chunk and tripped the HAM gate.
                    can_pop = len(pend_proj) >= (2 if b + 1 < B else 1)
                    oproj = pend_proj.pop(0) if can_pop else None
                    if oproj:
                        oSb = obp.tile([128, 8, 512], bf16, tag="o")
                        step = max(1, nkb // 4)
                        fill_at = {step - 1 + i * step: i for i in range(4)}

                    # lookahead 3: the diagonal-block causal masks add
                    # exp->gpsimd latency; at depth 2 the short (j=0)
                    # chunks stalled on them
                    sq = [emit_exp(j, kb0, *emit_S(b, j, qch, kb0))
                          for kb0 in range(min(3, nkb))]
                    for kb in range(nkb):
                        p, vstart = sq.pop(0)
                        pv = p.rearrange("p (h q) -> p h q", q=512)
                        gkb = b * NKB_B + kb
                        for h in range(2):
                            nc.tensor.matmul(psY[0:66, h, vstart:],
                                             vprv[:, gkb, h, :],
                                             pv[:, h, vstart:],
                                             start=(kb == 0), stop=(kb == nkb - 1))
                        if oproj and kb in fill_at:
                            if pend_odma:
                                pend_odma.pop(0)()
                            od = emit_proj_group(oproj[0], oproj[1], oSb,
                                                 fill_at[kb])
                            if od:
                                pend_odma.append(od)
                        if kb + 3 < nkb:
                            sq.append(emit_exp(j, kb + 3,
                                               *emit_S(b, j, qch, kb + 3)))

                    ep = emit_epilogue(qch, psY)
                    # prefetch x for the NEXT iteration's QKV chunk
                    if b + 1 < B:
                        nxt = b * CHB + CHB + j + 1
                        if nxt < NCH:
                            pend_x.append(emit_A_load(nxt))
                    # boundary: next batch's QKV, V transposes, and the
                    # deferred norm of chunk qch-1
                    xab = pend_x.pop(0) if b + 1 < B else None
                    if xab:
                        emit_A_mm_qk(b * CHB + CHB + j, *xab)
                    at = (emit_A_mm_v(b * CHB + CHB + j, *xab)
                          if xab else None)
                    if at:
                        emit_A_transp(*at)
                    if pend_norm:
                        pq, psbY, prrt = pend_norm.pop(0)
                        pend_proj.append((pq, emit_norm(pq, psbY, prrt)))
                    pend_norm.append((qch, *ep))
            # drain
            for od in pend_odma:
                od()
            pend_odma = []
            while pend_proj:
                emit_proj(*pend_proj.pop(0))
            for pq, psbY, prrt in pend_norm:
                emit_proj(pq, emit_norm(pq, psbY, prrt))

    nc.compile()
    return nc


def _get_program():
    if "nc" not in _CACHE:
        _CACHE["nc"] = _build_program()
    return _CACHE["nc"]


def kernel(x, w_attn, b_attn, w_proj, b_proj, _trace=False):
    import ml_dtypes
    from concourse.bass_utils import run_bass_kernel_spmd

    bf16 = ml_dtypes.bfloat16
    nc = _get_program()

    x = np.asarray(x, dtype=np.float32)
    w_attn = np.asarray(w_attn, dtype=np.float32)
    b_attn = np.asarray(b_attn, dtype=np.float32)
    w_proj = np.asarray(w_proj, dtype=np.float32)
    b_proj = np.asarray(b_proj, dtype=np.float32)

    f8 = ml_dtypes.float8_e4m3  # IEEE-style e4m3: matches TRN FP8_EXP4
    xTf = x.reshape(TOK, C).T
    xT_np = np.ascontiguousarray(xTf.astype(bf16))
    xT8_np = np.ascontiguousarray(xTf.astype(f8))

    in_maps = []
    for c in range(NCORES):
        lo, hi = c * 128, (c + 1) * 128
        wq = w_attn[:, lo:hi]
        wk = w_attn[:, C + lo:C + hi]
        wv = w_attn[:, 2 * C + lo:2 * C + hi]
        # q,k weights x16 into fp8 (w~0.02*N(0,1) would land in e4m3's
        # subnormal range unscaled); exp scale folds the 256 back out
        wqk8_np = np.ascontiguousarray(
            (np.concatenate([wq, wk], axis=1) * 16.0).astype(f8))
        wqkv_np = np.ascontiguousarray(wv.astype(bf16))
        bq = b_attn[lo:hi] * 16.0
        bk = b_attn[C + lo:C + hi] * 16.0
        bv = b_attn[2 * C + lo:2 * C + hi]
        battn_np = np.ascontiguousarray(np.stack([bq, bk, bv], axis=1))  # [128, 3]
        wproj_np = np.ascontiguousarray(w_proj[lo:hi, :].astype(bf16))
        if c == 0:
            bproj_np = np.ascontiguousarray(b_proj.reshape(8, 128).T)
        else:
            bproj_np = np.zeros((128, 8), dtype=np.float32)
        in_maps.append({
            "xT": xT_np,
            "xT8": xT8_np,
            "wqk8": wqk8_np,
            "wqkv": wqkv_np,
            "battn": battn_np,
            "wproj": wproj_np,
            "bproj": bproj_np,
        })

    res = run_bass_kernel_spmd(nc, in_maps, core_ids=list(range(NCORES)),
                               trace=_trace)
    acc = res.results[0]["outT"].astype(np.float32)
    for c in range(1, NCORES):
        acc += res.results[c]["outT"].astype(np.float32)
    out = np.ascontiguousarray(acc.T).reshape(B, T, C)
    if _trace:
        kernel.last_exec_time_ns = res.exec_time_ns
        kernel.last_scope_times = res.per_core_scope_times
        kernel.last_trace = res.instructions_and_trace
    return out
